# revision 20
# baseline (speedup 1.0000x reference)
"""CSWin Transformer block kernel for 8 Trainium2 NeuronCores.

Data-parallel over batch: 32 images -> 4 per core. Each core runs the full
block (LN1, qkv, cross-shaped window attention with LePE, proj, residual,
LN2, MLP, residual) on its shard, fully pipelined per image.

Layouts per core (T = 4*3136 = 12544 tokens):
  - token-major: (112 tokens on partitions, 128 ch free), 28 tiles per image.
  - channel-major: (128 ch on partitions, tokens free).
  - Branch 0 (56x2 column stripes) tokens are kept in w-major order
    (p = 56*w + h) in rows 0:64 of channel-major tensors; branch 1 rows
    64:128 use h-major (t = 56*h + w). Window w of either branch is then
    columns [112*w, 112*w+112).

LN gammas folded into the following matmul weights host-side; LN betas enter
as constant rows via per-partition bias adds on C-major evacuations.
LePE conv bias + the v-bias row are folded into the proj bias.
"""
import sys
sys.path.insert(0, "/opt/trn_rl_repo")
import os
import numpy as np
import concourse.bass as bass
from concourse import bacc
import concourse.tile as tile
from concourse import mybir
from concourse.bass_utils import run_bass_kernel_spmd
from concourse.masks import make_identity

F32 = mybir.dt.float32
BF16 = mybir.dt.bfloat16
AL = mybir.AluOpType
AF = mybir.ActivationFunctionType

N_CORES = 8
B, RESO, C = 32, 56, 128
L = RESO * RESO            # 3136
IMG = B // N_CORES         # 4 images per core
T = IMG * L                # 12544 tokens per core
PT = 112                   # tokens per token-major tile
NTI = L // PT              # 28 token tiles per image
CK = 448                   # tokens per C-major chunk
NCK = L // CK              # 7 chunks per image
NWIN = 28                  # windows per image per branch
WT = 112                   # tokens per window
HD = 32
EPS = 1e-5


def build(nc, dbg=()):
    x_in = nc.declare_dram_parameter("x", [T, C], F32, isOutput=False)
    wqkv_in = nc.declare_dram_parameter("wqkv", [C, 3 * C], F32, isOutput=False)
    wproj_in = nc.declare_dram_parameter("wproj", [C, C], F32, isOutput=False)
    wfc1_in = nc.declare_dram_parameter("wfc1", [C, 4 * C], F32, isOutput=False)
    wfc2_in = nc.declare_dram_parameter("wfc2", [4 * C, C], F32, isOutput=False)
    # vecs cols: 0:s2q 1:s2k 2:s2v 3:projb 4:fc2b 5:eps 6..14:taps 15..18:fc1b
    vecs_in = nc.declare_dram_parameter("vecs", [C, 19], F32, isOutput=False)
    out_t = nc.declare_dram_parameter("out", [T, C], BF16, isOutput=True)
    dbg_outs = {}
    for name, shape in dbg:
        dbg_outs[name] = nc.declare_dram_parameter(name, shape, F32, isOutput=True)

    tc = tile.TileContext(nc)
    with tc:
        with (
            tc.tile_pool(name="consts", bufs=1) as consts,
            tc.tile_pool(name="glob", bufs=1) as glob,
            tc.tile_pool(name="pimg", bufs=2) as pimg,
            tc.tile_pool(name="small", bufs=2) as small,
            tc.tile_pool(name="psU", bufs=3, space="PSUM") as psU,
            tc.tile_pool(name="psT", bufs=2, space="PSUM") as psT,
            tc.tile_pool(name="dscr", bufs=2, space="DRAM") as dscr,
        ):
            _body(nc, consts, glob, pimg, small, psU, psT, dscr,
                  x_in, wqkv_in, wproj_in, wfc1_in, wfc2_in, vecs_in,
                  out_t, dbg_outs)
    return nc


def _body(nc, consts, glob, pimg, small, psU, psT, dscr,
          x_in, wqkv_in, wproj_in, wfc1_in, wfc2_in, vecs_in, out_t, dbg_outs):
    # ---------------- constants / weights ----------------
    identb = consts.tile([128, 128], BF16)
    make_identity(nc, identb[:])
    onesb = consts.tile([WT, 32], BF16)
    nc.vector.memset(onesb[:], 1.0)
    ones1 = consts.tile([C, 1], BF16)
    nc.vector.memset(ones1[:], 1.0)
    wqkv = consts.tile([C, 3 * C], BF16)
    nc.gpsimd.dma_start(out=wqkv[:], in_=wqkv_in[:])
    wproj = consts.tile([C, C], BF16)
    nc.gpsimd.dma_start(out=wproj[:], in_=wproj_in[:])
    wfc1 = consts.tile([C, 4 * C], BF16)
    nc.gpsimd.dma_start(out=wfc1[:], in_=wfc1_in[:])
    wfc2 = consts.tile([C, 4, C], BF16)
    nc.gpsimd.dma_start(out=wfc2[:], in_=wfc2_in.rearrange("(k p) o -> p k o", p=C))
    vecs = consts.tile([C, 19], F32)
    nc.sync.dma_start(out=vecs[:], in_=vecs_in[:])
    s2q, s2k, s2v = vecs[:, 0:1], vecs[:, 1:2], vecs[:, 2:3]
    projb, fc2b, epsv = vecs[:, 3:4], vecs[:, 4:5], vecs[:, 5:6]
    taps = [vecs[:, 6 + i:7 + i] for i in range(9)]
    fc1b = [vecs[:, 15 + h:16 + h] for h in range(4)]

    def phase_A(img):
        # x in token-major tiles (tile ti = tokens [112*ti, +112) of this image)
        # This copy only feeds LN1 (stats+apply) and frees early; phase_B
        # re-loads its own copy so the A(i+2) input DMA isn't gated on B(i).
        x_tm = pimg.tile([PT, NTI, C], F32, tag="xA")
        base_t = 0
        nc.sync.dma_start(
            out=x_tm[:, :, :],
            in_=x_in[img * L:(img + 1) * L].rearrange("(n p) c -> p n c", p=PT))

        # ---- LN1 stats + apply + transpose ----
        mvs = small.tile([PT, NTI, 2], F32, tag="mvs")
        rstd = small.tile([PT, NTI], F32, tag="rstd")
        lnx_h = pimg.tile([C, L], BF16, tag="lnx_h")
        lnx_w = pimg.tile([C, L], BF16, tag="lnx_w")
        for tg in range(NTI // 7):
            for ti in range(7 * tg, 7 * tg + 7):
                st = small.tile([PT, 6], F32, tag="bnst")
                nc.vector.bn_stats(out=st[:], in_=x_tm[:, base_t + ti, :])
                nc.vector.bn_aggr(out=mvs[:, ti, :], in_=st[:])
            gsl = bass.ds(7 * tg, 7)
            nc.scalar.activation(rstd[:, gsl], mvs[:, gsl, 1], AF.Ln,
                                 bias=epsv[0:PT, :])
            nc.scalar.activation(rstd[:, gsl], rstd[:, gsl], AF.Exp, scale=-0.5)
            for ti in range(7 * tg, 7 * tg + 7):
                z = small.tile([PT, C], BF16, tag="zt")
                nc.vector.tensor_scalar(out=z[:], in0=x_tm[:, base_t + ti, :],
                                        scalar1=mvs[:, ti, 0:1],
                                        scalar2=rstd[:, ti:ti + 1],
                                        op0=AL.subtract, op1=AL.mult)
                ztp = psT.tile([C, PT], BF16, tag="tp")
                nc.tensor.transpose(ztp[:], z[:], identb[0:PT, 0:PT])
                nc.vector.tensor_copy(lnx_h[:, ti * PT:(ti + 1) * PT], ztp[:])
        # w-major reorder: p = 56*w + h  <-  t = 56*h + w (chunked so qkv
        # chunk ck can start as soon as its columns exist)
        lnw_v = lnx_w.rearrange("p (w h) -> p w h", w=RESO)
        lnh_v = lnx_h.rearrange("p (h w) -> p h w", h=RESO).rearrange("p h w -> p w h")
        for ck in range(NCK):
            wv = bass.ds(8 * ck, 8)
            nc.gpsimd.tensor_copy(out=lnw_v[:, wv, :], in_=lnh_v[:, wv, :])

        # ---- qkv (col-packed: br0 from lnx_w -> rows 0:64, br1 from lnx_h) ----
        # vT is padded by one col on each side so the LePE shift copies can
        # read [-1, L+1) with plain 2D (fast-mode) access patterns.
        qT = pimg.tile([C, L], BF16, tag="qT", bufs=1)
        kT = pimg.tile([C, L], BF16, tag="kT", bufs=1)
        vTp = pimg.tile([C, 2 + L], BF16, tag="vT", bufs=1)
        vT = vTp[:, 1:1 + L]
        for ck in range(NCK):
            sl = bass.ts(ck, CK)
            pqk = psU.tile([C, 2, 512], F32, tag="u")
            pv = psU.tile([C, 2, 512], F32, tag="u")
            for half, src in ((0, lnx_w), (1, lnx_h)):
                hs = bass.ds(64 * half, 64)
                nc.tensor.matmul(pqk[hs, 0, 0:CK], wqkv[:, bass.ds(64 * half, 64)],
                                 src[:, sl], start=True, stop=True,
                                 tile_position=(0, 64 * half))
                nc.tensor.matmul(pqk[hs, 1, 0:CK], wqkv[:, bass.ds(C + 64 * half, 64)],
                                 src[:, sl], start=True, stop=True,
                                 tile_position=(0, 64 * half))
                nc.tensor.matmul(pv[hs, 0, 0:CK], wqkv[:, bass.ds(2 * C + 64 * half, 64)],
                                 src[:, sl], start=True, stop=True,
                                 tile_position=(0, 64 * half))
            nc.scalar.activation(qT[:, sl], pqk[:, 0, 0:CK], AF.Identity, bias=s2q)
            nc.scalar.activation(kT[:, sl], pqk[:, 1, 0:CK], AF.Identity, bias=s2k)
            nc.vector.tensor_scalar(out=vT[:, sl], in0=pv[:, 0, 0:CK], scalar1=s2v,
                                    scalar2=None, op0=AL.add)

        # ---- v_tm: token-major v, 2 window-pairs per psum round ----
        v_tm = pimg.tile([PT, NWIN, C], BF16, tag="v_tm")
        for s2_ in range(NWIN // 2):
            pvt = psU.tile([PT, 2, 512], F32, tag="u")
            for g in range(2):
                wsl = bass.ts(2 * s2_ + g, WT)
                nc.tensor.matmul(pvt[:, g, 0:64], lnx_w[:, wsl], wqkv[:, 2 * C:2 * C + 64],
                                 start=True, stop=True, tile_position=(0, 0))
                nc.tensor.matmul(pvt[:, g, 64:128], lnx_h[:, wsl],
                                 wqkv[:, 2 * C + 64:3 * C],
                                 start=True, stop=True, tile_position=(0, 0))
            nc.vector.tensor_copy(v_tm[:, bass.ds(2 * s2_, 2), :], pvt[:, :, 0:C])

        # ---- attention: per superchunk (4 windows), heads in pairs ----
        tmp_att = pimg.tile([C, L], BF16, tag="tmp_att", bufs=1)
        for s4 in range(NCK):
            pT_t = small.tile([WT, 4, 4 * WT], BF16, tag="pT")
            for hp in range(2):            # head pairs {0,1}, {2,3}
                sp = psU.tile([WT, 2, 512], F32, tag="u")
                for g in range(4):
                    wsl = bass.ts(4 * s4 + g, WT)
                    for hh in range(2):
                        h = 2 * hp + hh
                        hsl = bass.ds(32 * h, 32)
                        nc.tensor.matmul(sp[:, hh, bass.ds(112 * g, WT)],
                                         kT[hsl, wsl], qT[hsl, wsl],
                                         start=True, stop=True,
                                         tile_position=(32 * h, 0))
                nc.scalar.activation(pT_t[:, bass.ds(2 * hp, 2), :],
                                     sp[:, :, 0:4 * WT], AF.Exp)
            sa = psU.tile([C, 2, 512], F32, tag="u")
            sums = sa[:, 0, 0:CK]
            avp = sa[:, 1, 0:CK]
            for h in range(4):
                po = bass.ds(32 * h, 32)
                nc.tensor.matmul(sa[po, 0, 0:CK], onesb[:, 0:32], pT_t[:, h, :],
                                 start=True, stop=True, tile_position=(0, 32 * h))
                vsl = bass.ds(64 * (h // 2) + 32 * (h % 2), 32)
                for g in range(4):
                    nc.tensor.matmul(sa[po, 1, bass.ds(112 * g, WT)],
                                     v_tm[:, 4 * s4 + g, vsl],
                                     pT_t[:, h, bass.ts(g, WT)],
                                     start=True, stop=True,
                                     tile_position=(0, 32 * h))
            lg = small.tile([C, CK], F32, tag="lg")
            nc.scalar.activation(lg[:], sums, AF.Ln)
            rec = small.tile([C, CK], F32, tag="rec")
            nc.scalar.activation(rec[:], lg[:], AF.Exp, scale=-1.0)
            nc.vector.tensor_tensor(out=tmp_att[:, bass.ts(s4, CK)], in0=avp,
                                    in1=rec[:], op=AL.mult)

        # ---- LePE taps accumulate onto tmp_att (stripe view c,s,y,x) ----
        # scalar_tensor_tensor only has a 1x uop; tensor_scalar into a scratch
        # + tensor_tensor add runs at 2-4x. dx-shifted taps read vTp at a +-1
        # element offset (pad cols keep it in bounds); the stripe-seam columns
        # that pick up the neighbouring stripe's value are re-zeroed on the
        # scratch before the add.
        aa = tmp_att.rearrange("p (s y x) -> p s y x", s=NWIN, y=2)

        def tap(dy, dx):
            t = taps[(dy + 1) * 3 + (dx + 1)]
            s = pimg.tile([C, L], BF16, tag="tsc", bufs=1)
            sh4 = vTp[:, 1 + dx:1 + dx + L].rearrange("p (s y x) -> p s y x",
                                                      s=NWIN, y=2)
            s4 = s.rearrange("p (s y x) -> p s y x", s=NWIN, y=2)
            if dy == 0:
                o = tmp_att[:]
                i, sv, sm = vTp[:, 1 + dx:1 + dx + L], s[:], s4[:, :, :, :]
            else:
                ysrc, ydst = (1, 0) if dy == 1 else (0, 1)
                o = aa[:, :, ydst:ydst + 1, :]
                i = sh4[:, :, ysrc:ysrc + 1, :]
                sv = s4[:, :, ysrc:ysrc + 1, :]
                sm = sv
            nc.vector.tensor_scalar(out=sv, in0=i, scalar1=t, scalar2=None,
                                    op0=AL.mult)
            if dx == 1:
                nc.vector.memset(sm[:, :, :, 55:56], 0.0)
            elif dx == -1:
                nc.vector.memset(sm[:, :, :, 0:1], 0.0)
            nc.vector.tensor_tensor(out=o, in0=sv, in1=o, op=AL.add)
        for dy in (0, 1, -1):
            for dx in (0, 1, -1):
                tap(dy, dx)

        # ---- reorder to full h-major att tensor ----
        att_h = pimg.tile([C, L], BF16, tag="att_h")
        ah_v = att_h[0:64].rearrange("p (h w) -> p h w", h=RESO)
        ta_v = tmp_att[0:64].rearrange("p (w h) -> p w h", w=RESO).rearrange("p w h -> p h w")
        for ck in range(NCK):
            hv = bass.ds(8 * ck, 8)
            nc.gpsimd.tensor_copy(out=ah_v[:, hv, :], in_=ta_v[:, hv, :])
            nc.gpsimd.tensor_copy(out=att_h[64:128, bass.ts(ck, CK)],
                                  in_=tmp_att[64:128, bass.ts(ck, CK)])

        if "attT" in dbg_outs and img == 0:
            dc = small.tile([C, L], F32, tag="dbg")
            nc.vector.tensor_copy(dc[:], att_h[:])
            nc.sync.dma_start(out=dbg_outs["attT"], in_=dc[:])
        return att_h

    def phase_B(img, state):
        """C-major back half: no PE transposes, no per-tile token-major ops.

        x arrives C-major via a casting gpsimd DMA + xbar-transpose DMAs; the
        residual stream stays [C, L] bf16. LN2 stats come from ones-matmuls
        (channel sums land in PSUM rows), the per-token scalars are broadcast
        back to [C, L] rows by DMA, and the output leaves via xbar-transpose
        DMAs straight from C-major."""
        att_h = state
        xb16 = pimg.tile([PT, NTI, C], BF16, tag="xb16", bufs=1)
        nc.gpsimd.dma_start(
            out=xb16[:, :, :],
            in_=x_in[img * L:(img + 1) * L].rearrange("(n p) c -> p n c", p=PT))
        xbC = pimg.tile([C, L], BF16, tag="xbC", bufs=1)
        for ti in range(NTI):
            nc.sync.dma_start(out=xbC[:, bass.ts(ti, PT)], in_=xb16[:, ti, :],
                              transpose=True)

        # ---- proj + bias + x residual fused into the PSUM evacuation ----
        x2C = pimg.tile([C, L], BF16, tag="x2C", bufs=1)
        for ck in range(NCK):
            sl = bass.ts(ck, CK)
            pp = psU.tile([C, 2, 512], F32, tag="u")
            nc.tensor.matmul(pp[:, 0, 0:CK], wproj[:], att_h[:, sl], start=True, stop=True)
            nc.vector.scalar_tensor_tensor(out=x2C[:, sl], in0=pp[:, 0, 0:CK],
                                           scalar=projb, in1=xbC[:, sl],
                                           op0=AL.add, op1=AL.add)

        # ---- LN2 stats: channel sums / sumsq via ones-matmuls ----
        # (DMA cannot read PSUM, so the [1, 2, 448] psum rows go through a
        # single-lane ACT copy into a bf16 row buffer. The row -> token-tile
        # [112, 28] reshape is a transpose; SBUF APs cannot split partitions
        # out of the free dim, so bounce through a padded DRAM (k, j, p) image
        # and use the DMA xbar-transpose back into SBUF.)
        rows = small.tile([1, 2, L], BF16, tag="rowsB", bufs=1)
        for ck in range(NCK):
            sl = bass.ts(ck, CK)
            sq = small.tile([C, CK], BF16, tag="sqB")
            nc.scalar.activation(sq[:], x2C[:, sl], AF.Square)
            pr = psU.tile([C, 2, 512], F32, tag="u")
            nc.tensor.matmul(pr[0:1, 0, 0:CK], ones1[:], x2C[:, sl],
                             start=True, stop=True)
            nc.tensor.matmul(pr[0:1, 1, 0:CK], ones1[:], sq[:],
                             start=True, stop=True)
            nc.scalar.activation(rows[0:1, :, sl], pr[0:1, 0:2, 0:CK],
                                 AF.Identity)
        rowsd = dscr.tile([2, 32, 128], BF16, tag="rowsd")
        nc.sync.dma_start(out=rowsd[:, 0:NTI, 0:PT], in_=rows[0:1, :, :])
        st_sb = small.tile([128, 2, 32], BF16, tag="st2")
        for k in range(2):
            nc.sync.dma_start(out=st_sb[:, k, :], in_=rowsd[k, :, :],
                              transpose=True)

        # rstd2 = 1/sqrt(sumsq/C - (sum1/C)^2 + eps); mean2 = sum1/C
        mean2 = small.tile([PT, NTI], F32, tag="mean2")
        var2 = small.tile([PT, NTI], F32, tag="var2")
        rstd2 = small.tile([PT, NTI], F32, tag="rstd2")
        nc.vector.tensor_scalar(out=mean2[:], in0=st_sb[0:PT, 0, 0:NTI],
                                scalar1=1.0 / C, scalar2=None, op0=AL.mult)
        nc.vector.tensor_tensor(out=var2[:], in0=mean2[:], in1=mean2[:], op=AL.mult)
        nc.vector.scalar_tensor_tensor(out=var2[:], in0=st_sb[0:PT, 1, 0:NTI],
                                       scalar=1.0 / C, in1=var2[:],
                                       op0=AL.mult, op1=AL.subtract)
        nc.scalar.activation(rstd2[:], var2[:], AF.Ln, bias=epsv[0:PT, :])
        nc.scalar.activation(rstd2[:], rstd2[:], AF.Exp, scale=-0.5)
        mr2 = small.tile([PT, NTI], F32, tag="mr2")
        nc.vector.tensor_tensor(out=mr2[:], in0=mean2[:], in1=rstd2[:], op=AL.mult)

        # ---- broadcast per-token scalars to [C, L] rows; apply LN2 ----
        # token-tile [112, 28] -> row order via pad + xbar transpose + gather;
        # then replicate across partitions by log-doubling DMA copies (SBUF
        # sources cannot have partition-stride 0).
        r2bp = small.tile([PT, 2, 128], BF16, tag="r2bp")
        nc.vector.memset(r2bp[:, :, :], 0.0)
        nc.vector.tensor_copy(r2bp[:, 0, 0:NTI], rstd2[:])
        nc.vector.tensor_copy(r2bp[:, 1, 0:NTI], mr2[:])
        rj = small.tile([128, 2, PT], BF16, tag="rj")
        for k in range(2):
            nc.sync.dma_start(out=rj[:, k, :], in_=r2bp[:, k, :], transpose=True)
        R2B = pimg.tile([C, L], BF16, tag="R2B", bufs=1)
        M2B = pimg.tile([C, L], BF16, tag="M2B", bufs=1)
        for buf, k in ((R2B, 0), (M2B, 1)):
            nc.sync.dma_start(out=buf[0:1, :], in_=rj[0:NTI, k, :])
            p = 1
            while p < C:
                nc.sync.dma_start(out=buf[p:2 * p, :], in_=buf[0:p, :])
                p *= 2
        lnx2 = pimg.tile([C, L], BF16, tag="lnx2", bufs=1)
        nc.vector.tensor_tensor(out=lnx2[:], in0=x2C[:], in1=R2B[:], op=AL.mult)
        nc.vector.tensor_tensor(out=lnx2[:], in0=lnx2[:], in1=M2B[:], op=AL.subtract)

        # ---- MLP; residual 2 fused into evacuation; output via xbar DMAs ----
        x3p = None
        for ck in range(NCK):
            sl = bass.ts(ck, CK)
            hb = small.tile([C, 4, CK], BF16, tag="hb")
            for hp in range(2):
                ph = psU.tile([C, 2, 512], F32, tag="u")
                for hh in range(2):
                    h = 2 * hp + hh
                    nc.tensor.matmul(ph[:, hh, 0:CK], wfc1[:, bass.ds(128 * h, 128)],
                                     lnx2[:, sl], start=True, stop=True)
                    nc.scalar.activation(hb[:, h, :], ph[:, hh, 0:CK], AF.Gelu,
                                         bias=fc1b[h])
            p2 = psU.tile([C, 2, 512], F32, tag="u")
            for h in range(4):
                nc.tensor.matmul(p2[:, 0, 0:CK], wfc2[:, h, :], hb[:, h, :],
                                 start=(h == 0), stop=(h == 3))
            if ck % 2 == 0:
                x3p = pimg.tile([C, 1024], BF16, tag="x3p")
                if ck == 6:
                    nc.vector.memset(x3p[:, CK:512], 0.0)
            off = (ck % 2) * CK
            nc.vector.scalar_tensor_tensor(out=x3p[:, bass.ds(off, CK)],
                                           in0=p2[:, 0, 0:CK], scalar=fc2b,
                                           in1=x2C[:, sl], op0=AL.add, op1=AL.add)
            if ck % 2 == 1 or ck == 6:
                ntile = 7 if ck < 6 else 4
                xo = pimg.tile([128, 7, C], BF16, tag="xo")
                for j in range(ntile):
                    nc.sync.dma_start(out=xo[:, j, :],
                                      in_=x3p[:, bass.ds(128 * j, 128)],
                                      transpose=True)
                base = img * L + 896 * (ck // 2)
                if ck < 6:
                    nc.sync.dma_start(
                        out=out_t[base:base + 896].rearrange("(j p) c -> p j c", p=128),
                        in_=xo[:, 0:7, :])
                else:
                    nc.sync.dma_start(
                        out=out_t[base:base + 384].rearrange("(j p) c -> p j c", p=128),
                        in_=xo[:, 0:3, :])
                    nc.sync.dma_start(
                        out=out_t[base + 384:base + 448].rearrange("(j p) c -> p j c", p=64),
                        in_=xo[0:64, 3:4, :])

    # Skewed software pipeline: emit A(i) then B(i-1) so each engine's
    # in-order stream interleaves independent work from adjacent images.
    state = [None] * IMG
    for i in range(IMG + 1):
        if i < IMG:
            state[i] = phase_A(i)
        if i >= 1:
            phase_B(i - 1, state[i - 1])


def _prep_inputs(inputs):
    """Host-side weight preprocessing (fp64 for exact folds)."""
    g1 = inputs["norm1_g"].astype(np.float64)
    b1 = inputs["norm1_b"].astype(np.float64)
    g2 = inputs["norm2_g"].astype(np.float64)
    b2 = inputs["norm2_b"].astype(np.float64)
    qkv_w = inputs["qkv_w"].astype(np.float64)
    proj_w = inputs["proj_w"].astype(np.float64)
    fc1_w = inputs["fc1_w"].astype(np.float64)
    fc2_w = inputs["fc2_w"].astype(np.float64)
    scale = HD ** -0.5

    wqkv = g1[:, None] * qkv_w
    s2 = b1 @ qkv_w
    wqkv[:, 0:C] *= scale
    s2q = s2[0:C] * scale
    s2k = s2[C:2 * C]
    s2v = s2[2 * C:3 * C]

    # LePE taps in stripe coords (y = stripe row in {0,1}, x = along stripe):
    # br1 (rows 64:128, h-major): (y,x) = (img_y, img_x) -> w1[dy+1, dx+1]
    # br0 (rows 0:64, w-major):  (y,x) = (img_x, img_y) -> transposed kernel
    w0 = inputs["conv_w0"].astype(np.float64)[:, 0]
    w1 = inputs["conv_w1"].astype(np.float64)[:, 0]
    taps = np.zeros((C, 9))
    for dy in (-1, 0, 1):
        for dx in (-1, 0, 1):
            ti = (dy + 1) * 3 + (dx + 1)
            taps[0:64, ti] = w0[:, dx + 1, dy + 1]
            taps[64:128, ti] = w1[:, dy + 1, dx + 1]

    cb = np.concatenate([inputs["conv_b0"], inputs["conv_b1"]]).astype(np.float64)
    projb_eff = inputs["proj_b"].astype(np.float64) + (s2v + cb) @ proj_w

    wfc1 = g2[:, None] * fc1_w
    fc1b_eff = b2 @ fc1_w + inputs["fc1_b"].astype(np.float64)

    vecs = np.zeros((C, 19))
    vecs[:, 0], vecs[:, 1], vecs[:, 2] = s2q, s2k, s2v
    vecs[:, 3], vecs[:, 4] = projb_eff, inputs["fc2_b"].astype(np.float64)
    vecs[:, 5] = EPS
    vecs[:, 6:15] = taps
    for h in range(4):
        vecs[:, 15 + h] = fc1b_eff[128 * h:128 * (h + 1)]

    return {
        "wqkv": np.ascontiguousarray(wqkv, np.float32),
        "wproj": np.ascontiguousarray(proj_w, np.float32),
        "wfc1": np.ascontiguousarray(wfc1, np.float32),
        "wfc2": np.ascontiguousarray(fc2_w, np.float32),
        "vecs": np.ascontiguousarray(vecs, np.float32),
    }


_CACHE = {}


class _Bacc(bacc.Bacc):
    """Bacc with the combined Ln+Exp activation-table set preferred, so the
    attention's Exp/Ln/Exp sequence stays on one table (the default
    first-match ordering alternates exp_and_others / natural_log and inserts
    a table load per activation)."""

    def insert_act_table_loads(self):
        import concourse.mybir as _mb
        from concourse.hw_specs import get_activation_tables as _gat
        from concourse.bacc import _bass_rust as _br
        has_activation = any(
            isinstance(i, _mb.InstActivation)
            for b in self.main_func.blocks
            for i in b.instructions
        )
        if not has_activation:
            return
        tables = list(_gat(self.m.arch).items())
        # Keep list order (set ids are positional); strip Exp/Ln from the
        # sets that precede natural_log_exp_and_others so first-match picks
        # the combined set for both.
        out = []
        for name, fns in tables:
            if name == "natural_log_exp_and_others":
                out.append((name, fns))
                continue
            if name in ("exp_and_others", "natural_log"):
                fns = {f for f in fns
                       if getattr(f, "name", str(f)) not in ("Exp", "Ln")}
            out.append((name, fns))
        _br.insert_act_table_loads(self, out)


def _get_nc(dbg=()):
    key = tuple(dbg)
    if key not in _CACHE:
        nc = _Bacc()
        build(nc, dbg)
        nc.finalize()
        _CACHE[key] = nc
    return _CACHE[key]


def kernel(**inputs):
    nc = _get_nc(_DBG[0] if _DBG else ())
    w = _prep_inputs(inputs)
    x = np.asarray(inputs["x"], np.float32)
    in_maps = []
    for c in range(N_CORES):
        m = dict(w)
        m["x"] = np.ascontiguousarray(x[c * IMG:(c + 1) * IMG].reshape(T, C))
        in_maps.append(m)
    trace = os.environ.get("KER_TRACE", "0") == "1"
    r = run_bass_kernel_spmd(nc, in_maps, list(range(N_CORES)), trace=trace)
    out = np.concatenate([np.asarray(r.results[c]["out"], np.float32).reshape(IMG, L, C)
                          for c in range(N_CORES)], axis=0)
    kernel.last_results = r
    return out


_DBG = []



# revision 23
# speedup vs baseline: 1.2323x; 1.2323x over previous
"""CSWin Transformer block kernel for 8 Trainium2 NeuronCores.

Data-parallel over batch: 32 images -> 4 per core. Each core runs the full
block (LN1, qkv, cross-shaped window attention with LePE, proj, residual,
LN2, MLP, residual) on its shard, fully pipelined per image.

Layouts per core (T = 4*3136 = 12544 tokens):
  - token-major: (112 tokens on partitions, 128 ch free), 28 tiles per image.
  - channel-major: (128 ch on partitions, tokens free).
  - Branch 0 (56x2 column stripes) tokens are kept in w-major order
    (p = 56*w + h) in rows 0:64 of channel-major tensors; branch 1 rows
    64:128 use h-major (t = 56*h + w). Window w of either branch is then
    columns [112*w, 112*w+112).

LN gammas folded into the following matmul weights host-side; LN betas enter
as constant rows via per-partition bias adds on C-major evacuations.
LePE conv bias + the v-bias row are folded into the proj bias.
"""
import sys
sys.path.insert(0, "/opt/trn_rl_repo")
import os
import numpy as np
import concourse.bass as bass
from concourse import bacc
import concourse.tile as tile
from concourse import mybir
from concourse.bass_utils import run_bass_kernel_spmd
from concourse.masks import make_identity

F32 = mybir.dt.float32
BF16 = mybir.dt.bfloat16
AL = mybir.AluOpType
AF = mybir.ActivationFunctionType

N_CORES = 8
B, RESO, C = 32, 56, 128
L = RESO * RESO            # 3136
IMG = B // N_CORES         # 4 images per core
T = IMG * L                # 12544 tokens per core
PT = 112                   # tokens per token-major tile
NTI = L // PT              # 28 token tiles per image
CK = 448                   # tokens per C-major chunk
NCK = L // CK              # 7 chunks per image
NWIN = 28                  # windows per image per branch
WT = 112                   # tokens per window
HD = 32
EPS = 1e-5


def build(nc, dbg=()):
    x_in = nc.declare_dram_parameter("x", [T, C], F32, isOutput=False)
    wqkv_in = nc.declare_dram_parameter("wqkv", [C, 3 * C], F32, isOutput=False)
    wproj_in = nc.declare_dram_parameter("wproj", [C, C], F32, isOutput=False)
    wfc1_in = nc.declare_dram_parameter("wfc1", [C, 4 * C], F32, isOutput=False)
    wfc2_in = nc.declare_dram_parameter("wfc2", [4 * C, C], F32, isOutput=False)
    # vecs cols: 0:s2q 1:s2k 2:s2v 3:projb 4:fc2b 5:eps 6..14:taps 15..18:fc1b
    vecs_in = nc.declare_dram_parameter("vecs", [C, 19], F32, isOutput=False)
    out_t = nc.declare_dram_parameter("out", [T, C], BF16, isOutput=True)
    dbg_outs = {}
    for name, shape in dbg:
        dbg_outs[name] = nc.declare_dram_parameter(name, shape, F32, isOutput=True)

    tc = tile.TileContext(nc)
    with tc:
        with (
            tc.tile_pool(name="consts", bufs=1) as consts,
            tc.tile_pool(name="glob", bufs=1) as glob,
            tc.tile_pool(name="pimg", bufs=2) as pimg,
            tc.tile_pool(name="small", bufs=2) as small,
            tc.tile_pool(name="psU", bufs=3, space="PSUM") as psU,
            tc.tile_pool(name="psT", bufs=2, space="PSUM") as psT,
            tc.tile_pool(name="dscr", bufs=2, space="DRAM") as dscr,
        ):
            _body(nc, consts, glob, pimg, small, psU, psT, dscr,
                  x_in, wqkv_in, wproj_in, wfc1_in, wfc2_in, vecs_in,
                  out_t, dbg_outs)
    return nc


def _body(nc, consts, glob, pimg, small, psU, psT, dscr,
          x_in, wqkv_in, wproj_in, wfc1_in, wfc2_in, vecs_in, out_t, dbg_outs):
    # ---------------- constants / weights ----------------
    identb = consts.tile([128, 128], BF16)
    make_identity(nc, identb[:])
    onesb = consts.tile([WT, 32], BF16)
    nc.vector.memset(onesb[:], 1.0)
    ones1 = consts.tile([C, 1], BF16)
    nc.vector.memset(ones1[:], 1.0)
    wqkv = consts.tile([C, 3 * C], BF16)
    nc.gpsimd.dma_start(out=wqkv[:], in_=wqkv_in[:])
    wproj = consts.tile([C, C], BF16)
    nc.gpsimd.dma_start(out=wproj[:], in_=wproj_in[:])
    wfc1 = consts.tile([C, 4 * C], BF16)
    nc.gpsimd.dma_start(out=wfc1[:], in_=wfc1_in[:])
    wfc2 = consts.tile([C, 4, C], BF16)
    nc.gpsimd.dma_start(out=wfc2[:], in_=wfc2_in.rearrange("(k p) o -> p k o", p=C))
    vecs = consts.tile([C, 19], F32)
    nc.sync.dma_start(out=vecs[:], in_=vecs_in[:])
    s2q, s2k, s2v = vecs[:, 0:1], vecs[:, 1:2], vecs[:, 2:3]
    projb, fc2b, epsv = vecs[:, 3:4], vecs[:, 4:5], vecs[:, 5:6]
    taps = [vecs[:, 6 + i:7 + i] for i in range(9)]
    fc1b = [vecs[:, 15 + h:16 + h] for h in range(4)]

    def phase_A(img):
        # x in token-major tiles (tile ti = tokens [112*ti, +112) of this image)
        # This copy only feeds LN1 (stats+apply) and frees early; phase_B
        # re-loads its own copy so the A(i+2) input DMA isn't gated on B(i).
        x_tm = pimg.tile([PT, NTI, C], F32, tag="xA")
        base_t = 0
        nc.sync.dma_start(
            out=x_tm[:, :, :],
            in_=x_in[img * L:(img + 1) * L].rearrange("(n p) c -> p n c", p=PT))

        # ---- LN1 stats + apply + transpose ----
        mvs = small.tile([PT, NTI, 2], F32, tag="mvs")
        rstd = small.tile([PT, NTI], F32, tag="rstd")
        lnx_h = pimg.tile([C, L], BF16, tag="lnx_h")
        lnx_w = pimg.tile([C, L], BF16, tag="lnx_w")
        zbuf = []

        def flush_z():
            g = zbuf[0][0] // 4
            ptz = psT.tile([128, 4, 128], BF16, tag="tpb")
            for ti, z in zbuf:
                nc.tensor.transpose(ptz[:, ti % 4, 0:PT], z[:], identb[0:PT, 0:PT])
            nc.vector.tensor_copy(lnx_h[:, bass.ts(g, CK)], ptz[:, :, 0:PT])
            zbuf.clear()

        for tg in range(NTI // 7):
            for ti in range(7 * tg, 7 * tg + 7):
                st = small.tile([PT, 6], F32, tag="bnst")
                nc.vector.bn_stats(out=st[:], in_=x_tm[:, base_t + ti, :])
                nc.vector.bn_aggr(out=mvs[:, ti, :], in_=st[:])
            gsl = bass.ds(7 * tg, 7)
            nc.scalar.activation(rstd[:, gsl], mvs[:, gsl, 1], AF.Ln,
                                 bias=epsv[0:PT, :])
            nc.scalar.activation(rstd[:, gsl], rstd[:, gsl], AF.Exp, scale=-0.5)
            for ti in range(7 * tg, 7 * tg + 7):
                z = small.tile([PT, C], BF16, tag="zt", bufs=12)
                nc.vector.tensor_scalar(out=z[:], in0=x_tm[:, base_t + ti, :],
                                        scalar1=mvs[:, ti, 0:1],
                                        scalar2=rstd[:, ti:ti + 1],
                                        op0=AL.subtract, op1=AL.mult)
                zbuf.append((ti, z))
                if len(zbuf) == 4:
                    flush_z()
        # w-major reorder: p = 56*w + h  <-  t = 56*h + w (chunked so qkv
        # chunk ck can start as soon as its columns exist)
        lnw_v = lnx_w.rearrange("p (w h) -> p w h", w=RESO)
        lnh_v = lnx_h.rearrange("p (h w) -> p h w", h=RESO).rearrange("p h w -> p w h")
        for ck in range(NCK):
            wv = bass.ds(8 * ck, 8)
            nc.gpsimd.tensor_copy(out=lnw_v[:, wv, :], in_=lnh_v[:, wv, :])

        # ---- qkv (col-packed: br0 from lnx_w -> rows 0:64, br1 from lnx_h) ----
        # vT is padded by one col on each side so the LePE shift copies can
        # read [-1, L+1) with plain 2D (fast-mode) access patterns.
        qT = pimg.tile([C, L], BF16, tag="qT", bufs=1)
        kT = pimg.tile([C, L], BF16, tag="kT", bufs=1)
        vTp = pimg.tile([C, 2 + L], BF16, tag="vT", bufs=1)
        vT = vTp[:, 1:1 + L]
        for ck in range(NCK):
            sl = bass.ts(ck, CK)
            pqk = psU.tile([C, 2, 512], F32, tag="u")
            pv = psU.tile([C, 2, 512], F32, tag="u")
            for half, src in ((0, lnx_w), (1, lnx_h)):
                hs = bass.ds(64 * half, 64)
                nc.tensor.matmul(pqk[hs, 0, 0:CK], wqkv[:, bass.ds(64 * half, 64)],
                                 src[:, sl], start=True, stop=True,
                                 tile_position=(0, 64 * half))
                nc.tensor.matmul(pqk[hs, 1, 0:CK], wqkv[:, bass.ds(C + 64 * half, 64)],
                                 src[:, sl], start=True, stop=True,
                                 tile_position=(0, 64 * half))
                nc.tensor.matmul(pv[hs, 0, 0:CK], wqkv[:, bass.ds(2 * C + 64 * half, 64)],
                                 src[:, sl], start=True, stop=True,
                                 tile_position=(0, 64 * half))
            nc.scalar.activation(qT[:, sl], pqk[:, 0, 0:CK], AF.Identity, bias=s2q)
            nc.scalar.activation(kT[:, sl], pqk[:, 1, 0:CK], AF.Identity, bias=s2k)
            nc.vector.tensor_scalar(out=vT[:, sl], in0=pv[:, 0, 0:CK], scalar1=s2v,
                                    scalar2=None, op0=AL.add)

        # ---- v_tm: token-major v, 2 window-pairs per psum round ----
        v_tm = pimg.tile([PT, NWIN, C], BF16, tag="v_tm")
        for s2_ in range(NWIN // 2):
            pvt = psU.tile([PT, 2, 512], F32, tag="u")
            for g in range(2):
                wsl = bass.ts(2 * s2_ + g, WT)
                nc.tensor.matmul(pvt[:, g, 0:64], lnx_w[:, wsl], wqkv[:, 2 * C:2 * C + 64],
                                 start=True, stop=True, tile_position=(0, 0))
                nc.tensor.matmul(pvt[:, g, 64:128], lnx_h[:, wsl],
                                 wqkv[:, 2 * C + 64:3 * C],
                                 start=True, stop=True, tile_position=(0, 0))
            nc.vector.tensor_copy(v_tm[:, bass.ds(2 * s2_, 2), :], pvt[:, :, 0:C])

        # ---- attention: per superchunk (4 windows), heads in pairs ----
        tmp_att = pimg.tile([C, L], BF16, tag="tmp_att", bufs=1)
        for s4 in range(NCK):
            pT_t = small.tile([WT, 4, 4 * WT], BF16, tag="pT")
            for hp in range(2):            # head pairs {0,1}, {2,3}
                sp = psU.tile([WT, 2, 512], F32, tag="u")
                for g in range(4):
                    wsl = bass.ts(4 * s4 + g, WT)
                    for hh in range(2):
                        h = 2 * hp + hh
                        hsl = bass.ds(32 * h, 32)
                        nc.tensor.matmul(sp[:, hh, bass.ds(112 * g, WT)],
                                         kT[hsl, wsl], qT[hsl, wsl],
                                         start=True, stop=True,
                                         tile_position=(32 * h, 0))
                nc.scalar.activation(pT_t[:, bass.ds(2 * hp, 2), :],
                                     sp[:, :, 0:4 * WT], AF.Exp)
            sa = psU.tile([C, 2, 512], F32, tag="u")
            sums = sa[:, 0, 0:CK]
            avp = sa[:, 1, 0:CK]
            for h in range(4):
                po = bass.ds(32 * h, 32)
                nc.tensor.matmul(sa[po, 0, 0:CK], onesb[:, 0:32], pT_t[:, h, :],
                                 start=True, stop=True, tile_position=(0, 32 * h))
                vsl = bass.ds(64 * (h // 2) + 32 * (h % 2), 32)
                for g in range(4):
                    nc.tensor.matmul(sa[po, 1, bass.ds(112 * g, WT)],
                                     v_tm[:, 4 * s4 + g, vsl],
                                     pT_t[:, h, bass.ts(g, WT)],
                                     start=True, stop=True,
                                     tile_position=(0, 32 * h))
            lg = small.tile([C, CK], F32, tag="lg")
            nc.scalar.activation(lg[:], sums, AF.Ln)
            rec = small.tile([C, CK], F32, tag="rec")
            nc.scalar.activation(rec[:], lg[:], AF.Exp, scale=-1.0)
            nc.vector.tensor_tensor(out=tmp_att[:, bass.ts(s4, CK)], in0=avp,
                                    in1=rec[:], op=AL.mult)

        # ---- LePE taps accumulate onto tmp_att (stripe view c,s,y,x) ----
        # scalar_tensor_tensor only has a 1x uop; tensor_scalar into a scratch
        # + tensor_tensor add runs at 2-4x. dx-shifted taps read vTp at a +-1
        # element offset (pad cols keep it in bounds); the stripe-seam columns
        # that pick up the neighbouring stripe's value are re-zeroed on the
        # scratch before the add.
        aa = tmp_att.rearrange("p (s y x) -> p s y x", s=NWIN, y=2)

        def tap(dy, dx):
            t = taps[(dy + 1) * 3 + (dx + 1)]
            s = pimg.tile([C, L], BF16, tag="tsc", bufs=1)
            sh4 = vTp[:, 1 + dx:1 + dx + L].rearrange("p (s y x) -> p s y x",
                                                      s=NWIN, y=2)
            s4 = s.rearrange("p (s y x) -> p s y x", s=NWIN, y=2)
            if dy == 0:
                o = tmp_att[:]
                i, sv, sm = vTp[:, 1 + dx:1 + dx + L], s[:], s4[:, :, :, :]
            else:
                ysrc, ydst = (1, 0) if dy == 1 else (0, 1)
                o = aa[:, :, ydst:ydst + 1, :]
                i = sh4[:, :, ysrc:ysrc + 1, :]
                sv = s4[:, :, ysrc:ysrc + 1, :]
                sm = sv
            nc.vector.tensor_scalar(out=sv, in0=i, scalar1=t, scalar2=None,
                                    op0=AL.mult)
            if dx == 1:
                nc.vector.memset(sm[:, :, :, 55:56], 0.0)
            elif dx == -1:
                nc.vector.memset(sm[:, :, :, 0:1], 0.0)
            nc.vector.tensor_tensor(out=o, in0=sv, in1=o, op=AL.add)
        for dy in (0, 1, -1):
            for dx in (0, 1, -1):
                tap(dy, dx)

        # ---- reorder to full h-major att tensor ----
        att_h = pimg.tile([C, L], BF16, tag="att_h")
        ah_v = att_h[0:64].rearrange("p (h w) -> p h w", h=RESO)
        ta_v = tmp_att[0:64].rearrange("p (w h) -> p w h", w=RESO).rearrange("p w h -> p h w")
        for ck in range(NCK):
            hv = bass.ds(8 * ck, 8)
            nc.gpsimd.tensor_copy(out=ah_v[:, hv, :], in_=ta_v[:, hv, :])
            nc.gpsimd.tensor_copy(out=att_h[64:128, bass.ts(ck, CK)],
                                  in_=tmp_att[64:128, bass.ts(ck, CK)])

        if "attT" in dbg_outs and img == 0:
            dc = small.tile([C, L], F32, tag="dbg")
            nc.vector.tensor_copy(dc[:], att_h[:])
            nc.sync.dma_start(out=dbg_outs["attT"], in_=dc[:])
        return att_h

    def phase_B(img, state):
        """C-major back half: no PE transposes, no per-tile token-major ops.

        x arrives C-major via a casting gpsimd DMA + xbar-transpose DMAs; the
        residual stream stays [C, L] bf16. LN2 stats come from ones-matmuls
        (channel sums land in PSUM rows), the per-token scalars are broadcast
        back to [C, L] rows by DMA, and the output leaves via xbar-transpose
        DMAs straight from C-major."""
        att_h = state
        xb16 = pimg.tile([PT, NTI, C], BF16, tag="xb16", bufs=1)
        nc.gpsimd.dma_start(
            out=xb16[:, :, :],
            in_=x_in[img * L:(img + 1) * L].rearrange("(n p) c -> p n c", p=PT))
        # The DMA xbar transpose path only runs at ~23 GB/s, so bulk
        # transposes go through the PE (grouped psum evacuations).
        xbC = pimg.tile([C, L], BF16, tag="xbC", bufs=1)
        for g in range(NCK):
            ptx = psT.tile([128, 4, 128], BF16, tag="tpb")
            for tj in range(4):
                nc.tensor.transpose(ptx[:, tj, 0:PT], xb16[:, 4 * g + tj, :],
                                    identb[0:PT, 0:PT])
            nc.vector.tensor_copy(xbC[:, bass.ts(g, CK)], ptx[:, :, 0:PT])

        # ---- proj + bias + x residual fused into the PSUM evacuation ----
        x2C = pimg.tile([C, L], BF16, tag="x2C", bufs=1)
        for ck in range(NCK):
            sl = bass.ts(ck, CK)
            pp = psU.tile([C, 2, 512], F32, tag="u")
            nc.tensor.matmul(pp[:, 0, 0:CK], wproj[:], att_h[:, sl], start=True, stop=True)
            nc.vector.scalar_tensor_tensor(out=x2C[:, sl], in0=pp[:, 0, 0:CK],
                                           scalar=projb, in1=xbC[:, sl],
                                           op0=AL.add, op1=AL.add)

        # ---- LN2 stats: channel sums / sumsq via ones-matmuls ----
        # (DMA cannot read PSUM, so the [1, 2, 448] psum rows go through a
        # single-lane ACT copy into a bf16 row buffer. The row -> token-tile
        # [112, 28] reshape is a transpose; SBUF APs cannot split partitions
        # out of the free dim, so bounce through a padded DRAM (k, j, p) image
        # and use the DMA xbar-transpose back into SBUF.)
        rows = small.tile([1, 2, L], BF16, tag="rowsB", bufs=1)
        for ck in range(NCK):
            sl = bass.ts(ck, CK)
            sq = small.tile([C, CK], BF16, tag="sqB")
            nc.scalar.activation(sq[:], x2C[:, sl], AF.Square)
            pr = psU.tile([C, 2, 512], F32, tag="u")
            nc.tensor.matmul(pr[0:1, 0, 0:CK], ones1[:], x2C[:, sl],
                             start=True, stop=True)
            nc.tensor.matmul(pr[0:1, 1, 0:CK], ones1[:], sq[:],
                             start=True, stop=True)
            nc.scalar.activation(rows[0:1, :, sl], pr[0:1, 0:2, 0:CK],
                                 AF.Identity)
        rowsd = dscr.tile([2, 32, 128], BF16, tag="rowsd")
        nc.sync.dma_start(out=rowsd[:, 0:NTI, 0:PT], in_=rows[0:1, :, :])
        st_sb = small.tile([128, 2, 32], BF16, tag="st2")
        for k in range(2):
            nc.sync.dma_start(out=st_sb[:, k, :], in_=rowsd[k, :, :],
                              transpose=True)

        # rstd2 = 1/sqrt(sumsq/C - (sum1/C)^2 + eps); mean2 = sum1/C
        mean2 = small.tile([PT, NTI], F32, tag="mean2")
        var2 = small.tile([PT, NTI], F32, tag="var2")
        rstd2 = small.tile([PT, NTI], F32, tag="rstd2")
        nc.vector.tensor_scalar(out=mean2[:], in0=st_sb[0:PT, 0, 0:NTI],
                                scalar1=1.0 / C, scalar2=None, op0=AL.mult)
        nc.vector.tensor_tensor(out=var2[:], in0=mean2[:], in1=mean2[:], op=AL.mult)
        nc.vector.scalar_tensor_tensor(out=var2[:], in0=st_sb[0:PT, 1, 0:NTI],
                                       scalar=1.0 / C, in1=var2[:],
                                       op0=AL.mult, op1=AL.subtract)
        nc.scalar.activation(rstd2[:], var2[:], AF.Ln, bias=epsv[0:PT, :])
        nc.scalar.activation(rstd2[:], rstd2[:], AF.Exp, scale=-0.5)
        mr2 = small.tile([PT, NTI], F32, tag="mr2")
        nc.vector.tensor_tensor(out=mr2[:], in0=mean2[:], in1=rstd2[:], op=AL.mult)

        # ---- broadcast per-token scalars to [C, L] rows; apply LN2 ----
        # token-tile [112, 28] -> row order via pad + xbar transpose + gather;
        # then replicate across partitions by log-doubling DMA copies (SBUF
        # sources cannot have partition-stride 0).
        r2bp = small.tile([PT, 2, 128], BF16, tag="r2bp")
        nc.vector.memset(r2bp[:, :, :], 0.0)
        nc.vector.tensor_copy(r2bp[:, 0, 0:NTI], rstd2[:])
        nc.vector.tensor_copy(r2bp[:, 1, 0:NTI], mr2[:])
        rj = small.tile([128, 2, PT], BF16, tag="rj")
        for k in range(2):
            nc.sync.dma_start(out=rj[:, k, :], in_=r2bp[:, k, :], transpose=True)
        R2B = pimg.tile([C, L], BF16, tag="R2B", bufs=1)
        M2B = pimg.tile([C, L], BF16, tag="M2B", bufs=1)
        for buf, k in ((R2B, 0), (M2B, 1)):
            nc.sync.dma_start(out=buf[0:1, :], in_=rj[0:NTI, k, :])
            p = 1
            while p < C:
                nc.sync.dma_start(out=buf[p:2 * p, :], in_=buf[0:p, :])
                p *= 2
        lnx2 = pimg.tile([C, L], BF16, tag="lnx2", bufs=1)
        nc.vector.tensor_tensor(out=lnx2[:], in0=x2C[:], in1=R2B[:], op=AL.mult)
        nc.vector.tensor_tensor(out=lnx2[:], in0=lnx2[:], in1=M2B[:], op=AL.subtract)

        # ---- MLP; residual 2 fused into evacuation; out via PE transposes ----
        out_sb = pimg.tile([PT, NTI, C], BF16, tag="osb", bufs=1)
        for ck in range(NCK):
            sl = bass.ts(ck, CK)
            hb = small.tile([C, 4, CK], BF16, tag="hb")
            for hp in range(2):
                ph = psU.tile([C, 2, 512], F32, tag="u")
                for hh in range(2):
                    h = 2 * hp + hh
                    nc.tensor.matmul(ph[:, hh, 0:CK], wfc1[:, bass.ds(128 * h, 128)],
                                     lnx2[:, sl], start=True, stop=True)
                    nc.scalar.activation(hb[:, h, :], ph[:, hh, 0:CK], AF.Gelu,
                                         bias=fc1b[h])
            p2 = psU.tile([C, 2, 512], F32, tag="u")
            for h in range(4):
                nc.tensor.matmul(p2[:, 0, 0:CK], wfc2[:, h, :], hb[:, h, :],
                                 start=(h == 0), stop=(h == 3))
            x3c = small.tile([C, CK], BF16, tag="x3c")
            nc.vector.scalar_tensor_tensor(out=x3c[:], in0=p2[:, 0, 0:CK],
                                           scalar=fc2b, in1=x2C[:, sl],
                                           op0=AL.add, op1=AL.add)
            pto = psT.tile([128, 4, 128], BF16, tag="tpb")
            for tj in range(4):
                nc.tensor.transpose(pto[0:PT, tj, :], x3c[:, bass.ts(tj, PT)],
                                    identb[:, 0:C])
            nc.scalar.activation(out_sb[:, bass.ds(4 * ck, 4), :],
                                 pto[0:PT, :, :], AF.Identity)
        nc.sync.dma_start(
            out=out_t[img * L:(img + 1) * L].rearrange("(n p) c -> p n c", p=PT),
            in_=out_sb[:, :, :])

    # Skewed software pipeline: emit A(i) then B(i-1) so each engine's
    # in-order stream interleaves independent work from adjacent images.
    state = [None] * IMG
    for i in range(IMG + 1):
        if i < IMG:
            state[i] = phase_A(i)
        if i >= 1:
            phase_B(i - 1, state[i - 1])


def _prep_inputs(inputs):
    """Host-side weight preprocessing (fp64 for exact folds)."""
    g1 = inputs["norm1_g"].astype(np.float64)
    b1 = inputs["norm1_b"].astype(np.float64)
    g2 = inputs["norm2_g"].astype(np.float64)
    b2 = inputs["norm2_b"].astype(np.float64)
    qkv_w = inputs["qkv_w"].astype(np.float64)
    proj_w = inputs["proj_w"].astype(np.float64)
    fc1_w = inputs["fc1_w"].astype(np.float64)
    fc2_w = inputs["fc2_w"].astype(np.float64)
    scale = HD ** -0.5

    wqkv = g1[:, None] * qkv_w
    s2 = b1 @ qkv_w
    wqkv[:, 0:C] *= scale
    s2q = s2[0:C] * scale
    s2k = s2[C:2 * C]
    s2v = s2[2 * C:3 * C]

    # LePE taps in stripe coords (y = stripe row in {0,1}, x = along stripe):
    # br1 (rows 64:128, h-major): (y,x) = (img_y, img_x) -> w1[dy+1, dx+1]
    # br0 (rows 0:64, w-major):  (y,x) = (img_x, img_y) -> transposed kernel
    w0 = inputs["conv_w0"].astype(np.float64)[:, 0]
    w1 = inputs["conv_w1"].astype(np.float64)[:, 0]
    taps = np.zeros((C, 9))
    for dy in (-1, 0, 1):
        for dx in (-1, 0, 1):
            ti = (dy + 1) * 3 + (dx + 1)
            taps[0:64, ti] = w0[:, dx + 1, dy + 1]
            taps[64:128, ti] = w1[:, dy + 1, dx + 1]

    cb = np.concatenate([inputs["conv_b0"], inputs["conv_b1"]]).astype(np.float64)
    projb_eff = inputs["proj_b"].astype(np.float64) + (s2v + cb) @ proj_w

    wfc1 = g2[:, None] * fc1_w
    fc1b_eff = b2 @ fc1_w + inputs["fc1_b"].astype(np.float64)

    vecs = np.zeros((C, 19))
    vecs[:, 0], vecs[:, 1], vecs[:, 2] = s2q, s2k, s2v
    vecs[:, 3], vecs[:, 4] = projb_eff, inputs["fc2_b"].astype(np.float64)
    vecs[:, 5] = EPS
    vecs[:, 6:15] = taps
    for h in range(4):
        vecs[:, 15 + h] = fc1b_eff[128 * h:128 * (h + 1)]

    return {
        "wqkv": np.ascontiguousarray(wqkv, np.float32),
        "wproj": np.ascontiguousarray(proj_w, np.float32),
        "wfc1": np.ascontiguousarray(wfc1, np.float32),
        "wfc2": np.ascontiguousarray(fc2_w, np.float32),
        "vecs": np.ascontiguousarray(vecs, np.float32),
    }


_CACHE = {}


class _Bacc(bacc.Bacc):
    """Bacc with the combined Ln+Exp activation-table set preferred, so the
    attention's Exp/Ln/Exp sequence stays on one table (the default
    first-match ordering alternates exp_and_others / natural_log and inserts
    a table load per activation)."""

    def insert_act_table_loads(self):
        import concourse.mybir as _mb
        from concourse.hw_specs import get_activation_tables as _gat
        from concourse.bacc import _bass_rust as _br
        has_activation = any(
            isinstance(i, _mb.InstActivation)
            for b in self.main_func.blocks
            for i in b.instructions
        )
        if not has_activation:
            return
        tables = list(_gat(self.m.arch).items())
        # Keep list order (set ids are positional); strip Exp/Ln from the
        # sets that precede natural_log_exp_and_others so first-match picks
        # the combined set for both.
        out = []
        for name, fns in tables:
            if name == "natural_log_exp_and_others":
                out.append((name, fns))
                continue
            if name in ("exp_and_others", "natural_log"):
                fns = {f for f in fns
                       if getattr(f, "name", str(f)) not in ("Exp", "Ln")}
            out.append((name, fns))
        _br.insert_act_table_loads(self, out)


def _get_nc(dbg=()):
    key = tuple(dbg)
    if key not in _CACHE:
        nc = _Bacc()
        build(nc, dbg)
        nc.finalize()
        _CACHE[key] = nc
    return _CACHE[key]


def kernel(**inputs):
    nc = _get_nc(_DBG[0] if _DBG else ())
    w = _prep_inputs(inputs)
    x = np.asarray(inputs["x"], np.float32)
    in_maps = []
    for c in range(N_CORES):
        m = dict(w)
        m["x"] = np.ascontiguousarray(x[c * IMG:(c + 1) * IMG].reshape(T, C))
        in_maps.append(m)
    trace = os.environ.get("KER_TRACE", "0") == "1"
    r = run_bass_kernel_spmd(nc, in_maps, list(range(N_CORES)), trace=trace)
    out = np.concatenate([np.asarray(r.results[c]["out"], np.float32).reshape(IMG, L, C)
                          for c in range(N_CORES)], axis=0)
    kernel.last_results = r
    return out


_DBG = []



# revision 26
# speedup vs baseline: 1.3249x; 1.0752x over previous
"""CSWin Transformer block kernel for 8 Trainium2 NeuronCores.

Data-parallel over batch: 32 images -> 4 per core. Each core runs the full
block (LN1, qkv, cross-shaped window attention with LePE, proj, residual,
LN2, MLP, residual) on its shard, fully pipelined per image.

Layouts per core (T = 4*3136 = 12544 tokens):
  - token-major: (112 tokens on partitions, 128 ch free), 28 tiles per image.
  - channel-major: (128 ch on partitions, tokens free).
  - Branch 0 (56x2 column stripes) tokens are kept in w-major order
    (p = 56*w + h) in rows 0:64 of channel-major tensors; branch 1 rows
    64:128 use h-major (t = 56*h + w). Window w of either branch is then
    columns [112*w, 112*w+112).

LN gammas folded into the following matmul weights host-side; LN betas enter
as constant rows via per-partition bias adds on C-major evacuations.
LePE conv bias + the v-bias row are folded into the proj bias.
"""
import sys
sys.path.insert(0, "/opt/trn_rl_repo")
import os
import numpy as np
import concourse.bass as bass
from concourse import bacc
import concourse.tile as tile
from concourse import mybir
from concourse.bass_utils import run_bass_kernel_spmd
from concourse.masks import make_identity

F32 = mybir.dt.float32
BF16 = mybir.dt.bfloat16
AL = mybir.AluOpType
AF = mybir.ActivationFunctionType

N_CORES = 8
B, RESO, C = 32, 56, 128
L = RESO * RESO            # 3136
IMG = B // N_CORES         # 4 images per core
T = IMG * L                # 12544 tokens per core
PT = 112                   # tokens per token-major tile
NTI = L // PT              # 28 token tiles per image
CK = 448                   # tokens per C-major chunk
NCK = L // CK              # 7 chunks per image
NWIN = 28                  # windows per image per branch
WT = 112                   # tokens per window
HD = 32
EPS = 1e-5


def build(nc, dbg=()):
    x_in = nc.declare_dram_parameter("x", [T, C], F32, isOutput=False)
    wqkv_in = nc.declare_dram_parameter("wqkv", [C, 3 * C], F32, isOutput=False)
    wproj_in = nc.declare_dram_parameter("wproj", [C, C], F32, isOutput=False)
    wfc1_in = nc.declare_dram_parameter("wfc1", [C, 4 * C], F32, isOutput=False)
    wfc2_in = nc.declare_dram_parameter("wfc2", [4 * C, C], F32, isOutput=False)
    # vecs cols: 0:s2q 1:s2k 2:s2v 3:projb 4:fc2b 5:eps 6..14:taps 15..18:fc1b
    vecs_in = nc.declare_dram_parameter("vecs", [C, 19], F32, isOutput=False)
    out_t = nc.declare_dram_parameter("out", [T, C], BF16, isOutput=True)
    dbg_outs = {}
    for name, shape in dbg:
        dbg_outs[name] = nc.declare_dram_parameter(name, shape, F32, isOutput=True)

    tc = tile.TileContext(nc)
    with tc:
        with (
            tc.tile_pool(name="consts", bufs=1) as consts,
            tc.tile_pool(name="glob", bufs=1) as glob,
            tc.tile_pool(name="pimg", bufs=2) as pimg,
            tc.tile_pool(name="small", bufs=2) as small,
            tc.tile_pool(name="psU", bufs=3, space="PSUM") as psU,
            tc.tile_pool(name="psT", bufs=2, space="PSUM") as psT,
            tc.tile_pool(name="dscr", bufs=2, space="DRAM") as dscr,
        ):
            _body(nc, consts, glob, pimg, small, psU, psT, dscr,
                  x_in, wqkv_in, wproj_in, wfc1_in, wfc2_in, vecs_in,
                  out_t, dbg_outs)
    return nc


def _body(nc, consts, glob, pimg, small, psU, psT, dscr,
          x_in, wqkv_in, wproj_in, wfc1_in, wfc2_in, vecs_in, out_t, dbg_outs):
    # ---------------- constants / weights ----------------
    identb = consts.tile([128, 128], BF16)
    make_identity(nc, identb[:])
    onesb = consts.tile([WT, 32], BF16)
    nc.vector.memset(onesb[:], 1.0)
    ones1 = consts.tile([C, 1], BF16)
    nc.vector.memset(ones1[:], 1.0)
    wqkv = consts.tile([C, 3 * C], BF16)
    nc.gpsimd.dma_start(out=wqkv[:], in_=wqkv_in[:])
    wproj = consts.tile([C, C], BF16)
    nc.gpsimd.dma_start(out=wproj[:], in_=wproj_in[:])
    wfc1 = consts.tile([C, 4 * C], BF16)
    nc.gpsimd.dma_start(out=wfc1[:], in_=wfc1_in[:])
    wfc2 = consts.tile([C, 4, C], BF16)
    nc.gpsimd.dma_start(out=wfc2[:], in_=wfc2_in.rearrange("(k p) o -> p k o", p=C))
    vecs = consts.tile([C, 19], F32)
    nc.sync.dma_start(out=vecs[:], in_=vecs_in[:])
    s2q, s2k, s2v = vecs[:, 0:1], vecs[:, 1:2], vecs[:, 2:3]
    projb, fc2b, epsv = vecs[:, 3:4], vecs[:, 4:5], vecs[:, 5:6]
    taps = [vecs[:, 6 + i:7 + i] for i in range(9)]
    fc1b = [vecs[:, 15 + h:16 + h] for h in range(4)]

    def phase_A(img):
        # x in token-major tiles (tile ti = tokens [112*ti, +112) of this image)
        # This copy only feeds LN1 (stats+apply) and frees early; phase_B
        # re-loads its own copy so the A(i+2) input DMA isn't gated on B(i).
        x_tm = pimg.tile([PT, NTI, C], F32, tag="xA")
        base_t = 0
        nc.sync.dma_start(
            out=x_tm[:, :, :],
            in_=x_in[img * L:(img + 1) * L].rearrange("(n p) c -> p n c", p=PT))

        # ---- LN1 stats + apply + transpose ----
        mvs = small.tile([PT, NTI, 2], F32, tag="mvs")
        rstd = small.tile([PT, NTI], F32, tag="rstd")
        lnx_h = pimg.tile([C, L], BF16, tag="lnx_h")
        lnx_w = pimg.tile([C, L], BF16, tag="lnx_w")
        zbuf = []

        def flush_z():
            g = zbuf[0][0] // 4
            ptz = psT.tile([128, 4, 128], BF16, tag="tpb")
            for ti, z in zbuf:
                nc.tensor.transpose(ptz[:, ti % 4, 0:PT], z[:], identb[0:PT, 0:PT])
            nc.vector.tensor_copy(lnx_h[:, bass.ts(g, CK)], ptz[:, :, 0:PT])
            zbuf.clear()

        for tg in range(NTI // 7):
            for ti in range(7 * tg, 7 * tg + 7):
                st = small.tile([PT, 6], F32, tag="bnst")
                nc.vector.bn_stats(out=st[:], in_=x_tm[:, base_t + ti, :])
                nc.vector.bn_aggr(out=mvs[:, ti, :], in_=st[:])
            gsl = bass.ds(7 * tg, 7)
            nc.scalar.activation(rstd[:, gsl], mvs[:, gsl, 1], AF.Ln,
                                 bias=epsv[0:PT, :])
            nc.scalar.activation(rstd[:, gsl], rstd[:, gsl], AF.Exp, scale=-0.5)
            for ti in range(7 * tg, 7 * tg + 7):
                z = small.tile([PT, C], BF16, tag="zt", bufs=12)
                nc.vector.tensor_scalar(out=z[:], in0=x_tm[:, base_t + ti, :],
                                        scalar1=mvs[:, ti, 0:1],
                                        scalar2=rstd[:, ti:ti + 1],
                                        op0=AL.subtract, op1=AL.mult)
                zbuf.append((ti, z))
                if len(zbuf) == 4:
                    flush_z()
        # w-major reorder: p = 56*w + h  <-  t = 56*h + w (chunked so qkv
        # chunk ck can start as soon as its columns exist)
        lnw_v = lnx_w.rearrange("p (w h) -> p w h", w=RESO)
        lnh_v = lnx_h.rearrange("p (h w) -> p h w", h=RESO).rearrange("p h w -> p w h")
        for ck in range(NCK):
            wv = bass.ds(8 * ck, 8)
            nc.gpsimd.tensor_copy(out=lnw_v[:, wv, :], in_=lnh_v[:, wv, :])

        # ---- qkv (col-packed: br0 from lnx_w -> rows 0:64, br1 from lnx_h) ----
        # vT is padded by one col on each side so the LePE shift copies can
        # read [-1, L+1) with plain 2D (fast-mode) access patterns.
        qT = pimg.tile([C, L], BF16, tag="qT", bufs=1)
        kT = pimg.tile([C, L], BF16, tag="kT", bufs=1)
        vTp = pimg.tile([C, 2 + L], BF16, tag="vT", bufs=1)
        vT = vTp[:, 1:1 + L]
        for ck in range(NCK):
            sl = bass.ts(ck, CK)
            pqk = psU.tile([C, 2, 512], F32, tag="u")
            pv = psU.tile([C, 2, 512], F32, tag="u")
            for half, src in ((0, lnx_w), (1, lnx_h)):
                hs = bass.ds(64 * half, 64)
                nc.tensor.matmul(pqk[hs, 0, 0:CK], wqkv[:, bass.ds(64 * half, 64)],
                                 src[:, sl], start=True, stop=True,
                                 tile_position=(0, 64 * half))
                nc.tensor.matmul(pqk[hs, 1, 0:CK], wqkv[:, bass.ds(C + 64 * half, 64)],
                                 src[:, sl], start=True, stop=True,
                                 tile_position=(0, 64 * half))
                nc.tensor.matmul(pv[hs, 0, 0:CK], wqkv[:, bass.ds(2 * C + 64 * half, 64)],
                                 src[:, sl], start=True, stop=True,
                                 tile_position=(0, 64 * half))
            nc.scalar.activation(qT[:, sl], pqk[:, 0, 0:CK], AF.Identity, bias=s2q)
            nc.scalar.activation(kT[:, sl], pqk[:, 1, 0:CK], AF.Identity, bias=s2k)
            nc.vector.tensor_scalar(out=vT[:, sl], in0=pv[:, 0, 0:CK], scalar1=s2v,
                                    scalar2=None, op0=AL.add)

        # ---- v_tm: token-major v, 2 window-pairs per psum round ----
        v_tm = pimg.tile([PT, NWIN, C], BF16, tag="v_tm")
        for s2_ in range(NWIN // 2):
            pvt = psU.tile([PT, 2, 512], F32, tag="u")
            for g in range(2):
                wsl = bass.ts(2 * s2_ + g, WT)
                nc.tensor.matmul(pvt[:, g, 0:64], lnx_w[:, wsl], wqkv[:, 2 * C:2 * C + 64],
                                 start=True, stop=True, tile_position=(0, 0))
                nc.tensor.matmul(pvt[:, g, 64:128], lnx_h[:, wsl],
                                 wqkv[:, 2 * C + 64:3 * C],
                                 start=True, stop=True, tile_position=(0, 0))
            nc.vector.tensor_copy(v_tm[:, bass.ds(2 * s2_, 2), :], pvt[:, :, 0:C])

        # ---- attention: per superchunk (4 windows), heads in pairs ----
        tmp_att = pimg.tile([C, L], BF16, tag="tmp_att", bufs=1)
        for s4 in range(NCK):
            pT_t = small.tile([WT, 4, 4 * WT], BF16, tag="pT")
            for hp in range(2):            # head pairs {0,1}, {2,3}
                sp = psU.tile([WT, 2, 512], F32, tag="u")
                for g in range(4):
                    wsl = bass.ts(4 * s4 + g, WT)
                    for hh in range(2):
                        h = 2 * hp + hh
                        hsl = bass.ds(32 * h, 32)
                        nc.tensor.matmul(sp[:, hh, bass.ds(112 * g, WT)],
                                         kT[hsl, wsl], qT[hsl, wsl],
                                         start=True, stop=True,
                                         tile_position=(32 * h, 0))
                nc.scalar.activation(pT_t[:, bass.ds(2 * hp, 2), :],
                                     sp[:, :, 0:4 * WT], AF.Exp)
            sa = psU.tile([C, 2, 512], F32, tag="u")
            sums = sa[:, 0, 0:CK]
            avp = sa[:, 1, 0:CK]
            for h in range(4):
                po = bass.ds(32 * h, 32)
                nc.tensor.matmul(sa[po, 0, 0:CK], onesb[:, 0:32], pT_t[:, h, :],
                                 start=True, stop=True, tile_position=(0, 32 * h))
                vsl = bass.ds(64 * (h // 2) + 32 * (h % 2), 32)
                for g in range(4):
                    nc.tensor.matmul(sa[po, 1, bass.ds(112 * g, WT)],
                                     v_tm[:, 4 * s4 + g, vsl],
                                     pT_t[:, h, bass.ts(g, WT)],
                                     start=True, stop=True,
                                     tile_position=(0, 32 * h))
            lg = small.tile([C, CK], F32, tag="lg", bufs=1)
            nc.scalar.activation(lg[:], sums, AF.Ln)
            rec = small.tile([C, CK], F32, tag="rec", bufs=1)
            nc.scalar.activation(rec[:], lg[:], AF.Exp, scale=-1.0)
            nc.vector.tensor_tensor(out=tmp_att[:, bass.ts(s4, CK)], in0=avp,
                                    in1=rec[:], op=AL.mult)

        # ---- LePE taps accumulate onto tmp_att (stripe view c,s,y,x) ----
        # scalar_tensor_tensor only has a 1x uop; tensor_scalar into a scratch
        # + tensor_tensor add runs at 2-4x. dx-shifted taps read vTp at a +-1
        # element offset (pad cols keep it in bounds); the stripe-seam columns
        # that pick up the neighbouring stripe's value are re-zeroed on the
        # scratch before the add.
        aa = tmp_att.rearrange("p (s y x) -> p s y x", s=NWIN, y=2)

        def tap(dy, dx):
            t = taps[(dy + 1) * 3 + (dx + 1)]
            s = pimg.tile([C, L], BF16, tag="tsc", bufs=1)
            sh4 = vTp[:, 1 + dx:1 + dx + L].rearrange("p (s y x) -> p s y x",
                                                      s=NWIN, y=2)
            s4 = s.rearrange("p (s y x) -> p s y x", s=NWIN, y=2)
            if dy == 0:
                o = tmp_att[:]
                i, sv, sm = vTp[:, 1 + dx:1 + dx + L], s[:], s4[:, :, :, :]
            else:
                ysrc, ydst = (1, 0) if dy == 1 else (0, 1)
                o = aa[:, :, ydst:ydst + 1, :]
                i = sh4[:, :, ysrc:ysrc + 1, :]
                sv = s4[:, :, ysrc:ysrc + 1, :]
                sm = sv
            nc.vector.tensor_scalar(out=sv, in0=i, scalar1=t, scalar2=None,
                                    op0=AL.mult)
            if dx == 1:
                nc.vector.memset(sm[:, :, :, 55:56], 0.0)
            elif dx == -1:
                nc.vector.memset(sm[:, :, :, 0:1], 0.0)
            nc.vector.tensor_tensor(out=o, in0=sv, in1=o, op=AL.add)
        for dy in (0, 1, -1):
            for dx in (0, 1, -1):
                tap(dy, dx)

        # ---- reorder to full h-major att tensor ----
        att_h = pimg.tile([C, L], BF16, tag="att_h")
        ah_v = att_h[0:64].rearrange("p (h w) -> p h w", h=RESO)
        ta_v = tmp_att[0:64].rearrange("p (w h) -> p w h", w=RESO).rearrange("p w h -> p h w")
        for ck in range(NCK):
            hv = bass.ds(8 * ck, 8)
            nc.gpsimd.tensor_copy(out=ah_v[:, hv, :], in_=ta_v[:, hv, :])
            nc.gpsimd.tensor_copy(out=att_h[64:128, bass.ts(ck, CK)],
                                  in_=tmp_att[64:128, bass.ts(ck, CK)])

        if "attT" in dbg_outs and img == 0:
            dc = small.tile([C, L], F32, tag="dbg")
            nc.vector.tensor_copy(dc[:], att_h[:])
            nc.sync.dma_start(out=dbg_outs["attT"], in_=dc[:])
        return att_h

    def phase_B1(img, state):
        """C-major back half: no PE transposes, no per-tile token-major ops.

        x arrives C-major via a casting gpsimd DMA + xbar-transpose DMAs; the
        residual stream stays [C, L] bf16. LN2 stats come from ones-matmuls
        (channel sums land in PSUM rows), the per-token scalars are broadcast
        back to [C, L] rows by DMA, and the output leaves via xbar-transpose
        DMAs straight from C-major."""
        att_h = state
        xb16 = pimg.tile([PT, NTI, C], BF16, tag="xb16", bufs=1)
        nc.gpsimd.dma_start(
            out=xb16[:, :, :],
            in_=x_in[img * L:(img + 1) * L].rearrange("(n p) c -> p n c", p=PT))
        # The DMA xbar transpose path only runs at ~23 GB/s, so bulk
        # transposes go through the PE (grouped psum evacuations).
        xbC = pimg.tile([C, L], BF16, tag="xbC", bufs=1)
        for g in range(NCK):
            ptx = psT.tile([128, 4, 128], BF16, tag="tpb")
            for tj in range(4):
                nc.tensor.transpose(ptx[:, tj, 0:PT], xb16[:, 4 * g + tj, :],
                                    identb[0:PT, 0:PT])
            nc.vector.tensor_copy(xbC[:, bass.ts(g, CK)], ptx[:, :, 0:PT])

        # ---- proj + bias + x residual fused into the PSUM evacuation ----
        x2C = pimg.tile([C, L], BF16, tag="x2C", bufs=2)
        for ck in range(NCK):
            sl = bass.ts(ck, CK)
            pp = psU.tile([C, 2, 512], F32, tag="u")
            nc.tensor.matmul(pp[:, 0, 0:CK], wproj[:], att_h[:, sl], start=True, stop=True)
            nc.vector.scalar_tensor_tensor(out=x2C[:, sl], in0=pp[:, 0, 0:CK],
                                           scalar=projb, in1=xbC[:, sl],
                                           op0=AL.add, op1=AL.add)

        # ---- LN2 stats: channel sums / sumsq via ones-matmuls ----
        # (DMA cannot read PSUM, so the [1, 2, 448] psum rows go through a
        # single-lane ACT copy into a bf16 row buffer. The row -> token-tile
        # [112, 28] reshape is a transpose; SBUF APs cannot split partitions
        # out of the free dim, so bounce through a padded DRAM (k, j, p) image
        # and use the DMA xbar-transpose back into SBUF.)
        rowsd = dscr.tile([2, 32, 128], BF16, tag="rowsd")
        for ck in range(NCK):
            sl = bass.ts(ck, CK)
            sq = small.tile([C, CK], BF16, tag="sqB", bufs=1)
            nc.scalar.activation(sq[:], x2C[:, sl], AF.Square)
            pr = psU.tile([C, 2, 512], F32, tag="u")
            nc.tensor.matmul(pr[0:1, 0, 0:CK], ones1[:], x2C[:, sl],
                             start=True, stop=True)
            nc.tensor.matmul(pr[0:1, 1, 0:CK], ones1[:], sq[:],
                             start=True, stop=True)
            rc = small.tile([1, 2, CK], BF16, tag="rowc")
            nc.scalar.activation(rc[0:1, :, :], pr[0:1, 0:2, 0:CK], AF.Identity)
            nc.sync.dma_start(out=rowsd[:, bass.ds(4 * ck, 4), 0:PT],
                              in_=rc[0:1, :, :])
        st_sb = small.tile([128, 2, 32], BF16, tag="st2")
        for k in range(2):
            nc.sync.dma_start(out=st_sb[:, k, :], in_=rowsd[k, :, :],
                              transpose=True)

        # rstd2 = 1/sqrt(sumsq/C - (sum1/C)^2 + eps); mean2 = sum1/C
        mean2 = small.tile([PT, NTI], F32, tag="mean2")
        var2 = small.tile([PT, NTI], F32, tag="var2")
        rstd2 = small.tile([PT, NTI], F32, tag="rstd2")
        nc.vector.tensor_scalar(out=mean2[:], in0=st_sb[0:PT, 0, 0:NTI],
                                scalar1=1.0 / C, scalar2=None, op0=AL.mult)
        nc.vector.tensor_tensor(out=var2[:], in0=mean2[:], in1=mean2[:], op=AL.mult)
        nc.vector.scalar_tensor_tensor(out=var2[:], in0=st_sb[0:PT, 1, 0:NTI],
                                       scalar=1.0 / C, in1=var2[:],
                                       op0=AL.mult, op1=AL.subtract)
        nc.scalar.activation(rstd2[:], var2[:], AF.Ln, bias=epsv[0:PT, :])
        nc.scalar.activation(rstd2[:], rstd2[:], AF.Exp, scale=-0.5)
        mr2 = small.tile([PT, NTI], F32, tag="mr2")
        nc.vector.tensor_tensor(out=mr2[:], in0=mean2[:], in1=rstd2[:], op=AL.mult)

        # ---- broadcast per-token scalars to [C, L] rows; apply LN2 ----
        # token-tile [112, 28] -> row order via pad + xbar transpose + gather;
        # then replicate across partitions by log-doubling DMA copies (SBUF
        # sources cannot have partition-stride 0).
        r2bp = small.tile([PT, 2, 128], BF16, tag="r2bp")
        nc.vector.memset(r2bp[:, :, :], 0.0)
        nc.vector.tensor_copy(r2bp[:, 0, 0:NTI], rstd2[:])
        nc.vector.tensor_copy(r2bp[:, 1, 0:NTI], mr2[:])
        rj = small.tile([128, 2, PT], BF16, tag="rj")
        for k in range(2):
            nc.sync.dma_start(out=rj[:, k, :], in_=r2bp[:, k, :], transpose=True)
        R2B = pimg.tile([C, L], BF16, tag="R2B", bufs=2)
        M2B = pimg.tile([C, L], BF16, tag="M2B", bufs=2)
        # R and M doubling chains on different DMA queues (sync / scalar)
        for buf, k, eng in ((R2B, 0, nc.sync), (M2B, 1, nc.scalar)):
            eng.dma_start(out=buf[0:1, :], in_=rj[0:NTI, k, :])
            p = 1
            while p < C:
                eng.dma_start(out=buf[p:2 * p, :], in_=buf[0:p, :])
                p *= 2
        return x2C, att_h, R2B, M2B

    def phase_B2(img, stateb):
        x2C, att_h, R2B, M2B = stateb
        lnx2 = pimg.tile([C, L], BF16, tag="lnx2", bufs=1)
        nc.vector.tensor_tensor(out=lnx2[:], in0=x2C[:], in1=R2B[:], op=AL.mult)
        nc.vector.tensor_tensor(out=lnx2[:], in0=lnx2[:], in1=M2B[:], op=AL.subtract)

        # ---- MLP; residual 2 fused into evacuation; out via PE transposes ----
        out_sb = pimg.tile([PT, NTI, C], BF16, tag="osb", bufs=1)
        for ck in range(NCK):
            sl = bass.ts(ck, CK)
            hb = small.tile([C, 4, CK], BF16, tag="hb")
            for hp in range(2):
                ph = psU.tile([C, 2, 512], F32, tag="u")
                for hh in range(2):
                    h = 2 * hp + hh
                    nc.tensor.matmul(ph[:, hh, 0:CK], wfc1[:, bass.ds(128 * h, 128)],
                                     lnx2[:, sl], start=True, stop=True)
                    nc.scalar.activation(hb[:, h, :], ph[:, hh, 0:CK], AF.Gelu,
                                         bias=fc1b[h])
            p2 = psU.tile([C, 2, 512], F32, tag="u")
            for h in range(4):
                nc.tensor.matmul(p2[:, 0, 0:CK], wfc2[:, h, :], hb[:, h, :],
                                 start=(h == 0), stop=(h == 3))
            x3c = small.tile([C, CK], BF16, tag="x3c")
            nc.vector.scalar_tensor_tensor(out=x3c[:], in0=p2[:, 0, 0:CK],
                                           scalar=fc2b, in1=x2C[:, sl],
                                           op0=AL.add, op1=AL.add)
            pto = psT.tile([128, 4, 128], BF16, tag="tpb")
            for tj in range(4):
                nc.tensor.transpose(pto[0:PT, tj, :], x3c[:, bass.ts(tj, PT)],
                                    identb[:, 0:C])
            nc.scalar.activation(out_sb[:, bass.ds(4 * ck, 4), :],
                                 pto[0:PT, :, :], AF.Identity)
        nc.sync.dma_start(
            out=out_t[img * L:(img + 1) * L].rearrange("(n p) c -> p n c", p=PT),
            in_=out_sb[:, :, :])

    # Skewed software pipeline, two-deep on the back half: the LN2-stats
    # row/broadcast latency chain of B1(i) is covered by A(i+2) and B2(i-1)
    # work that sits between B1(i) and B2(i) in emission order.
    state = [None] * IMG
    stateb = [None] * IMG
    for i in range(IMG + 2):
        if i < IMG:
            state[i] = phase_A(i)
        if 2 <= i:
            phase_B2(i - 2, stateb[i - 2])
        if 1 <= i <= IMG:
            stateb[i - 1] = phase_B1(i - 1, state[i - 1])


def _prep_inputs(inputs):
    """Host-side weight preprocessing (fp64 for exact folds)."""
    g1 = inputs["norm1_g"].astype(np.float64)
    b1 = inputs["norm1_b"].astype(np.float64)
    g2 = inputs["norm2_g"].astype(np.float64)
    b2 = inputs["norm2_b"].astype(np.float64)
    qkv_w = inputs["qkv_w"].astype(np.float64)
    proj_w = inputs["proj_w"].astype(np.float64)
    fc1_w = inputs["fc1_w"].astype(np.float64)
    fc2_w = inputs["fc2_w"].astype(np.float64)
    scale = HD ** -0.5

    wqkv = g1[:, None] * qkv_w
    s2 = b1 @ qkv_w
    wqkv[:, 0:C] *= scale
    s2q = s2[0:C] * scale
    s2k = s2[C:2 * C]
    s2v = s2[2 * C:3 * C]

    # LePE taps in stripe coords (y = stripe row in {0,1}, x = along stripe):
    # br1 (rows 64:128, h-major): (y,x) = (img_y, img_x) -> w1[dy+1, dx+1]
    # br0 (rows 0:64, w-major):  (y,x) = (img_x, img_y) -> transposed kernel
    w0 = inputs["conv_w0"].astype(np.float64)[:, 0]
    w1 = inputs["conv_w1"].astype(np.float64)[:, 0]
    taps = np.zeros((C, 9))
    for dy in (-1, 0, 1):
        for dx in (-1, 0, 1):
            ti = (dy + 1) * 3 + (dx + 1)
            taps[0:64, ti] = w0[:, dx + 1, dy + 1]
            taps[64:128, ti] = w1[:, dy + 1, dx + 1]

    cb = np.concatenate([inputs["conv_b0"], inputs["conv_b1"]]).astype(np.float64)
    projb_eff = inputs["proj_b"].astype(np.float64) + (s2v + cb) @ proj_w

    wfc1 = g2[:, None] * fc1_w
    fc1b_eff = b2 @ fc1_w + inputs["fc1_b"].astype(np.float64)

    vecs = np.zeros((C, 19))
    vecs[:, 0], vecs[:, 1], vecs[:, 2] = s2q, s2k, s2v
    vecs[:, 3], vecs[:, 4] = projb_eff, inputs["fc2_b"].astype(np.float64)
    vecs[:, 5] = EPS
    vecs[:, 6:15] = taps
    for h in range(4):
        vecs[:, 15 + h] = fc1b_eff[128 * h:128 * (h + 1)]

    return {
        "wqkv": np.ascontiguousarray(wqkv, np.float32),
        "wproj": np.ascontiguousarray(proj_w, np.float32),
        "wfc1": np.ascontiguousarray(wfc1, np.float32),
        "wfc2": np.ascontiguousarray(fc2_w, np.float32),
        "vecs": np.ascontiguousarray(vecs, np.float32),
    }


_CACHE = {}


class _Bacc(bacc.Bacc):
    """Bacc with the combined Ln+Exp activation-table set preferred, so the
    attention's Exp/Ln/Exp sequence stays on one table (the default
    first-match ordering alternates exp_and_others / natural_log and inserts
    a table load per activation)."""

    def insert_act_table_loads(self):
        import concourse.mybir as _mb
        from concourse.hw_specs import get_activation_tables as _gat
        from concourse.bacc import _bass_rust as _br
        has_activation = any(
            isinstance(i, _mb.InstActivation)
            for b in self.main_func.blocks
            for i in b.instructions
        )
        if not has_activation:
            return
        tables = list(_gat(self.m.arch).items())
        # Keep list order (set ids are positional); strip Exp/Ln from the
        # sets that precede natural_log_exp_and_others so first-match picks
        # the combined set for both.
        out = []
        for name, fns in tables:
            if name == "natural_log_exp_and_others":
                out.append((name, fns))
                continue
            if name in ("exp_and_others", "natural_log"):
                fns = {f for f in fns
                       if getattr(f, "name", str(f)) not in ("Exp", "Ln")}
            out.append((name, fns))
        _br.insert_act_table_loads(self, out)


def _get_nc(dbg=()):
    key = tuple(dbg)
    if key not in _CACHE:
        nc = _Bacc()
        build(nc, dbg)
        nc.finalize()
        _CACHE[key] = nc
    return _CACHE[key]


def kernel(**inputs):
    nc = _get_nc(_DBG[0] if _DBG else ())
    w = _prep_inputs(inputs)
    x = np.asarray(inputs["x"], np.float32)
    in_maps = []
    for c in range(N_CORES):
        m = dict(w)
        m["x"] = np.ascontiguousarray(x[c * IMG:(c + 1) * IMG].reshape(T, C))
        in_maps.append(m)
    trace = os.environ.get("KER_TRACE", "0") == "1"
    r = run_bass_kernel_spmd(nc, in_maps, list(range(N_CORES)), trace=trace)
    out = np.concatenate([np.asarray(r.results[c]["out"], np.float32).reshape(IMG, L, C)
                          for c in range(N_CORES)], axis=0)
    kernel.last_results = r
    return out


_DBG = []

